# revision 34
# baseline (speedup 1.0000x reference)
"""Trainium2 Bass kernel for nn_MultiHeadAttention_19396072309379.

Module math (per reference): all H=8 heads identical; V projected from `key`;
causal mask; softmax; concat of identical heads @ Wo  ==  o @ (sum of Wo row
blocks).  Computed as single-head attention with a reduced Wo.

Key simplifications vs a naive lowering:
  * bq/bk shift scores by a per-query constant -> softmax-invariant -> dropped.
    bv contributes bv @ Wo_r to every output row (attn rows sum to 1) -> folded
    into the host-side output bias.  No bias ops on-chip at all.
  * Causal/parity masks multiply the first (partial) 128-col block of each
    key-block's exp'd scores.  The per-j planes {tri | ones | zeros} are
    synthesized on-chip as tri * b_j + a_j from per-core selector scalars,
    which resolves the parity-dependent mask choice with a shared program.
  * V is computed directly row-major (out partitions = tokens), so no PE
    transpose / vT staging is needed; an extra ones-column in v1 accumulates
    softmax denominators for free inside the PV matmul.
  * Softmax normalization is deferred past the output projection: out rows
    are scaled by 1/denom in the PSUM->SBUF copy (DVE tensor_scalar / ACT
    activation-scale).  Denominators are transposed into per-partition layout
    with tiny PE row transposes.

Sharding: 8 cores = 4 batches x 2 "parity" halves.  Each core owns 8 of the
16 query blocks (128 rows each) of one batch, paired {i, 15-i} so causal work
is balanced.  Both parities run the SAME program with unified per-key-block
suffix widths (max over parities).

On-chip layout is fully transposed ("T" = [feature, seq]); all attention
operands are f16 (full-rate PE for any tile size, 2x/4x DVE modes), PSUM
accumulation f32.  K/V flow in 512-column chunks overlapped with input DMA.
"""

import numpy as np

B, S, D, H, DK, DV = 4, 2048, 512, 8, 64, 64
NB = S // 128  # 16 key/query blocks per batch
QB = 8  # query blocks per core
SQ = QB * 128  # 1024 query rows per core
N_CORES = 8

# per-parity query block sets (pairs {i, 15-i} -> equal causal work 68)
BLOCKS = {
    0: [0, 2, 4, 6, 9, 11, 13, 15],
    1: [1, 3, 5, 7, 8, 10, 12, 14],
}
# unified suffix width (in 128-blocks) for key-block j = max over parities of
# count of local query blocks with global index >= j
WIDTHS = [
    max(sum(1 for g in BLOCKS[p] if g >= j) for p in (0, 1)) for j in range(NB)
]

# PV accumulator regions: (col_lo, col_hi, psum_tile_idx, tile_col_off, last_j)
PO_R = [(0, 512, 0, 0, 7), (512, 896, 1, 0, 13), (896, 1024, 1, 384, 15)]
# output chunks per region (global 128-col chunk indices)
R_CHUNKS = [(0, 1, 2, 3), (4, 5, 6), (7,)]

# cst (f16) column layout: unduplicated 4-chunk Wq / Wk / Wv, then tri
C_WQ = 0
C_WK = 256
C_WV = 512
C_TRI = 768         # causal tri plane tri[ks, qs] = (qs >= ks)
C_NCOL = 896

N_WARM = 5  # PE p-state warm-up matmuls (512 cols each)


def _build(reps=1, dbg=False):
    import concourse.mybir as mybir
    import concourse.tile as tile
    from concourse import bacc

    F32 = mybir.dt.float32
    F16 = mybir.dt.float16

    nc = bacc.Bacc("TRN2", target_bir_lowering=False, debug=False, num_devices=N_CORES)
    d_qT = nc.dram_tensor("qT", [D, SQ], F16, kind="ExternalInput").ap()
    d_kT = nc.dram_tensor("kT", [D, S], F16, kind="ExternalInput").ap()
    d_cst = nc.dram_tensor("cst", [128, C_NCOL], F16, kind="ExternalInput").ap()
    d_wo = nc.dram_tensor("wo", [DV, D], F16, kind="ExternalInput").ap()
    d_msk = nc.dram_tensor("msk", [128, 2 * NB + 1], F32, kind="ExternalInput").ap()
    d_out = nc.dram_tensor("out", [SQ, D], F16, kind="ExternalOutput").ap()

    for _ in range(reps):
        _emit_body(nc, tile, mybir, d_qT, d_kT, d_cst, d_wo, d_msk, d_out)
    nc.compile()
    return nc


def _emit_body(nc, tile, mybir, d_qT, d_kT, d_cst, d_wo, d_msk, d_out):
    F32 = mybir.dt.float32
    F16 = mybir.dt.float16
    AF = mybir.ActivationFunctionType
    ALU = mybir.AluOpType

    with (
        tile.TileContext(nc) as tc,
        nc.allow_low_precision(reason="f16 attention kernel"),
    ):
        with (
            tc.tile_pool(name="const", bufs=1) as cpool,
            tc.tile_pool(name="acts", bufs=1) as apool,
            tc.tile_pool(name="work", bufs=3) as wpool,
            tc.tile_pool(name="outb", bufs=8) as opool,
            tc.tile_pool(name="psmisc", bufs=2, space="PSUM") as psm,
            tc.tile_pool(name="pscore", bufs=2, space="PSUM") as psc,
            tc.tile_pool(name="pacc", bufs=1, space="PSUM") as pacc,
        ):
            # ---- PE warm-up starts immediately (DVE memset -> zero matmuls)
            # so the modeled p-state ramp completes during the input DMAs ----
            warm = cpool.tile([128, 512], F16)
            nc.vector.memset(warm[:], 0.0)
            wps = psm.tile([128, 512], F32, tag="pp", name="wps")
            for _ in range(N_WARM):
                nc.tensor.matmul(
                    wps[:], warm[:, 0:128], warm[:],
                    start=True, stop=True, skip_group_check=True,
                )

            # ---- input DMAs (order = transfer order on the shared DMA
            # rsrc): Wq first (tiny), then q in 256-col chunks so the first
            # qproj starts early, then the other weights and the k chunks ----
            cst_t = cpool.tile([128, C_NCOL], F16)
            nc.sync.dma_start(cst_t[:, 0:256], d_cst[:, 0:256])
            qT_act = apool.tile([128, 4, SQ], F16)
            qT_r = d_qT.rearrange("(c p) s -> p c s", p=128)
            kT_r = d_kT.rearrange("(c p) s -> p c s", p=128)
            for n in range(4):
                sl = slice(256 * n, 256 * (n + 1))
                nc.sync.dma_start(qT_act[:, :, sl], qT_r[:, :, sl])
            msk_t = cpool.tile([128, 2 * NB + 1], F32)
            nc.sync.dma_start(msk_t[:], d_msk[:])
            nc.sync.dma_start(cst_t[:, 256:C_NCOL], d_cst[:, 256:C_NCOL])
            kT_act = apool.tile([128, 4, S], F16)
            nc.sync.dma_start(kT_act[:, :, 0:512], kT_r[:, :, 0:512])
            wo_t = cpool.tile([DV, D], F16)
            nc.sync.dma_start(wo_t[:], d_wo[:])
            for n in range(1, 4):
                sl = slice(512 * n, 512 * (n + 1))
                nc.sync.dma_start(kT_act[:, :, sl], kT_r[:, :, sl])

            id1 = msk_t[0:1, 2 * NB : 2 * NB + 1]
            # per-j mask planes {tri | ones | zeros} synthesized as
            # tri * b_j + a_j with per-core selector scalars; runs on idle
            # DVE time during the input-DMA window.
            tri_t = cst_t[:, C_TRI : C_TRI + 128]
            bm16 = apool.tile([128, NB, 128], F16)
            for j in range(NB):
                nc.vector.tensor_scalar(
                    bm16[:, j, :], tri_t[:],
                    msk_t[:, j : j + 1], msk_t[:, NB + j : NB + j + 1],
                    ALU.mult, ALU.add,
                )

            # ---- persistent SBUF tensors ----
            qT_proj = apool.tile([DK, SQ], F16)
            kT_proj = apool.tile([DK, S], F16)
            v1 = apool.tile([128, NB, DV + 1], F16)
            nc.vector.memset(v1[:, :, DV : DV + 1], 1.0)
            oT = apool.tile([DV, SQ], F16)
            srow = apool.tile([1, SQ], F32)
            recipT = apool.tile([128, QB], F32)
            po = [
                pacc.tile([DV + 1, 512], F32, name="po0"),
                pacc.tile([DV + 1, 512], F32, name="po12"),
            ]

            def qproj(n):
                sl = slice(256 * n, 256 * (n + 1))
                ps = psm.tile([128, 512], F32, tag="pp", name="psq")
                for c in range(4):
                    w = cst_t[:, C_WQ + 64 * c : C_WQ + 64 * c + 64]
                    nc.tensor.matmul(
                        ps[0:DK, 0:256], w, qT_act[:, c, sl],
                        start=(c == 0), stop=(c == 3),
                    )
                nc.scalar.copy(qT_proj[:, sl], ps[0:DK, 0:256])

            def kvproj(n):
                sl = slice(512 * n, 512 * (n + 1))
                ps = psm.tile([128, 512], F32, tag="pp", name="psk")
                for c in range(4):
                    w = cst_t[:, C_WK + 64 * c : C_WK + 64 * c + 64]
                    nc.tensor.matmul(
                        ps[0:DK, :], w, kT_act[:, c, sl],
                        start=(c == 0), stop=(c == 3),
                    )
                nc.vector.tensor_copy(kT_proj[:, sl], ps[0:DK, :])
                # v computed directly row-major: out partitions = tokens, so
                # no PE transpose / vT staging is needed.  4 token-blocks per
                # chunk; groups must stay contiguous (start=True marks the
                # tile's whole psum bank pending-zero).
                ps = psm.tile([128, 512], F32, tag="pp", name="psv")
                for jb in range(4):
                    tok = slice(512 * n + 128 * jb, 512 * n + 128 * jb + 128)
                    for c in range(4):
                        w = cst_t[:, C_WV + 64 * c : C_WV + 64 * c + 64]
                        nc.tensor.matmul(
                            ps[:, 64 * jb : 64 * jb + 64],
                            kT_act[:, c, tok], w[:],
                            start=(c == 0), stop=(c == 3),
                            skip_group_check=True,
                        )
                nc.vector.tensor_copy(v1[:, 4 * n : 4 * n + 4, 0:DV], ps[:, 0:256])

            exps = {}

            def scores(*js):
                # one psc tile + one exp activation shared by the (narrow)
                # key-blocks js; per-j mask multiplies on each first block
                ps_s = psc.tile([128, 1024], F32, tag="pss")
                expT = wpool.tile([128, 1024], F16, tag="expT", bufs=4)
                off = 0
                offs = []
                for j in js:
                    wblk = WIDTHS[j]
                    c0 = 128 * (QB - wblk)
                    cols = 128 * wblk
                    done = 0
                    while done < cols:
                        # matmul writes must not cross a psum bank boundary
                        nsz = min(512 - (off + done) % 512, cols - done)
                        nc.tensor.matmul(
                            ps_s[:, off + done : off + done + nsz],
                            kT_proj[:, 128 * j : 128 * (j + 1)],
                            qT_proj[:, c0 + done : c0 + done + nsz],
                            start=True, stop=True,
                        )
                        done += nsz
                    offs.append((j, off, cols))
                    off += cols
                nc.scalar.activation(
                    expT[:, 0:off], ps_s[:, 0:off], AF.Exp, bias=0.0, scale=0.125
                )
                for j, o, cols in offs:
                    nc.vector.tensor_mul(
                        expT[:, o : o + 128], expT[:, o : o + 128], bm16[:, j, :]
                    )
                    exps[j] = expT[:, o : o + cols]

            def emit_pv(j, masked):
                # masked=False: bulk columns (depend on exp only);
                # masked=True: the first 128-col block (awaits the mask mul).
                # start=True marks the tile's WHOLE 2KB psum bank pending-
                # zero, so only the first write of each bank carries it; the
                # bank-sharing region 2 and the masked block are zero-filled
                # by the pending state instead.
                wblk = WIDTHS[j]
                c0 = 128 * (QB - wblk)
                for rlo, rhi, ti, toff, lastj in PO_R:
                    lo = max(c0, rlo)
                    if lo >= rhi:
                        continue
                    if lo == c0:
                        a, b = (c0, c0 + 128) if masked else (c0 + 128, rhi)
                    else:
                        if masked:
                            continue
                        a, b = lo, rhi
                    if a >= b:
                        continue
                    nc.tensor.matmul(
                        po[ti][:, toff + a - rlo : toff + b - rlo],
                        v1[:, j, :],
                        exps[j][:, a - c0 : b - c0],
                        start=(j == 0 and toff == 0 and not masked),
                        stop=(j == lastj),
                        skip_group_check=True,
                    )

            def pv(j):
                emit_pv(j, False)
                emit_pv(j, True)

            def ep_copy(q):
                rlo, rhi, ti, toff, _ = PO_R[q]
                w = rhi - rlo
                nc.scalar.copy(oT[:, rlo:rhi], po[ti][0:DV, toff : toff + w])
                nc.vector.tensor_copy(
                    srow[:, rlo:rhi], po[ti][DV : DV + 1, toff : toff + w]
                )

            def ep_den(q):
                chunks = R_CHUNKS[q]
                pd = psm.tile([128, 512], F32, tag="pp", name="pd")
                for ci, g in enumerate(chunks):
                    nc.tensor.transpose(
                        pd[:, ci : ci + 1],
                        srow[0:1, 128 * g : 128 * (g + 1)],
                        id1,
                    )
                g0 = chunks[0]
                nc.vector.reciprocal(
                    recipT[:, g0 : g0 + len(chunks)], pd[:, 0 : len(chunks)]
                )

            def ep_out(q, i):
                g = R_CHUNKS[q][i]
                pf = psm.tile([128, 512], F32, tag="pp", name="pf")
                nc.tensor.matmul(
                    pf[:], oT[:, 128 * g : 128 * (g + 1)], wo_t[:],
                    start=True, stop=True,
                )
                osb = opool.tile([128, D], F16, tag="osb")
                if g % 2 == 0:
                    nc.vector.tensor_scalar_mul(osb[:], pf[:], recipT[:, g : g + 1])
                else:
                    nc.scalar.activation(
                        osb[:], pf[:], mybir.ActivationFunctionType.Copy,
                        bias=0.0, scale=recipT[:, g : g + 1],
                    )
                eng = nc.sync if g % 2 == 0 else nc.scalar
                eng.dma_start(d_out[128 * g : 128 * (g + 1), :], osb[:])

            # ---- schedule ----
            qproj(0)
            qproj(1)
            qproj(2)
            qproj(3)
            kvproj(0)
            kvproj(1)
            for j in range(8):
                if j == 4:
                    kvproj(2)
                scores(j)
                if j >= 1:
                    pv(j - 1)
            scores(8)
            scores(9)
            pv(7)
            ep_copy(0)
            kvproj(3)
            scores(10)
            scores(11)
            pv(8)
            pv(9)
            ep_den(0)
            ep_out(0, 0)
            scores(12)
            scores(13)
            pv(10)
            pv(11)
            ep_out(0, 1)
            ep_out(0, 2)
            scores(14, 15)
            pv(12)
            pv(13)
            ep_out(0, 3)
            ep_copy(1)
            ep_den(1)
            ep_out(1, 0)
            pv(14)
            ep_out(1, 1)
            pv(15)
            ep_copy(2)
            ep_den(2)
            ep_out(2, 0)
            ep_out(1, 2)


_NC_CACHE = None


def _get_nc():
    global _NC_CACHE
    if _NC_CACHE is None:
        _NC_CACHE = _build()
    return _NC_CACHE


def make_in_maps(query, key, Wq, Wk, Wv, Wo):
    query = np.asarray(query, dtype=np.float32)
    key = np.asarray(key, dtype=np.float32)
    Wq = np.asarray(Wq, dtype=np.float32)
    Wk = np.asarray(Wk, dtype=np.float32)
    Wv = np.asarray(Wv, dtype=np.float32)
    Wo = np.asarray(Wo, dtype=np.float32)

    wo_r = Wo.reshape(H, DV, D).sum(axis=0).astype(np.float16)  # [DV, D]
    cst = np.zeros((128, C_NCOL), np.float16)
    for w, base in ((Wq, C_WQ), (Wk, C_WK), (Wv, C_WV)):
        cst[:, base : base + 256] = (
            w.astype(np.float16).reshape(4, 128, 64).transpose(1, 0, 2).reshape(128, 256)
        )
    cst[:, C_TRI : C_TRI + 128] = np.triu(np.ones((128, 128), np.float16))

    in_maps = []
    for c in range(N_CORES):
        b, p = divmod(c, 2)
        blocks = BLOCKS[p]
        rows = np.concatenate(
            [np.arange(128 * g, 128 * (g + 1)) for g in blocks]
        )
        qT = np.ascontiguousarray(query[b][rows].T).astype(np.float16)
        kT = np.ascontiguousarray(key[b].T).astype(np.float16)
        msk = np.zeros((128, 2 * NB + 1), np.float32)
        for j in range(NB):
            g = blocks[QB - WIDTHS[j]]
            if g == j:
                msk[:, j] = 1.0  # b: tri plane
            elif g > j:
                msk[:, NB + j] = 1.0  # a: all-ones plane
            # g < j: both zero -> all-zeros plane
        msk[:, 2 * NB] = 1.0  # 1x1 identity for the denom row transposes
        in_maps.append({"qT": qT, "kT": kT, "cst": cst, "wo": wo_r, "msk": msk})
    return in_maps


def gather_output(results, bias_term):
    """results: list of per-core {'out': [SQ, D]}; adds host-folded bias."""
    out = np.empty((B, S, D), np.float32)
    for c in range(N_CORES):
        b, p = divmod(c, 2)
        blocks = BLOCKS[p]
        co = np.asarray(results[c]["out"], dtype=np.float32)
        for bp, g in enumerate(blocks):
            out[b, 128 * g : 128 * (g + 1), :] = co[128 * bp : 128 * (bp + 1), :]
    out += bias_term
    return out


def kernel(query, key, value, Wq, bq, Wk, bk, Wv, bv, Wo, bo):
    from concourse import bass_utils

    nc = _get_nc()
    in_maps = make_in_maps(query, key, Wq, Wk, Wv, Wo)
    res = bass_utils.run_bass_kernel_spmd(
        nc, in_maps, core_ids=list(range(N_CORES))
    )
    Wo = np.asarray(Wo, dtype=np.float32)
    wo_r = Wo.reshape(H, DV, D).sum(axis=0)
    # bq/bk only shift scores per query row (softmax-invariant); bv adds
    # bv @ Wo_r to every output row since attention rows sum to 1.
    bias_term = np.asarray(bv, np.float32) @ wo_r + np.asarray(bo, np.float32)
    return gather_output(res.results, bias_term.astype(np.float32))


# revision 37
# speedup vs baseline: 1.0181x; 1.0181x over previous
"""Trainium2 Bass kernel for nn_MultiHeadAttention_19396072309379.

Module math (per reference): all H=8 heads identical; V projected from `key`;
causal mask; softmax; concat of identical heads @ Wo  ==  o @ (sum of Wo row
blocks).  Computed as single-head attention with a reduced Wo.

Key simplifications vs a naive lowering:
  * bq/bk shift scores by a per-query constant -> softmax-invariant -> dropped.
    bv contributes bv @ Wo_r to every output row (attn rows sum to 1) -> folded
    into the host-side output bias.  No bias ops on-chip at all.
  * Causal/parity masks multiply the first (partial) 128-col block of each
    key-block's exp'd scores.  The per-j planes {tri | ones | zeros} are
    synthesized on-chip as tri * b_j + a_j from per-core selector scalars,
    which resolves the parity-dependent mask choice with a shared program.
  * V is computed directly row-major (out partitions = tokens), so no PE
    transpose / vT staging is needed; an extra ones-column in v1 accumulates
    softmax denominators for free inside the PV matmul.
  * Softmax normalization is deferred past the output projection: out rows
    are scaled by 1/denom in the PSUM->SBUF copy (DVE tensor_scalar / ACT
    activation-scale).  Denominators are transposed into per-partition layout
    with tiny PE row transposes.

Sharding: 8 cores = 4 batches x 2 "parity" halves.  Each core owns 8 of the
16 query blocks (128 rows each) of one batch, paired {i, 15-i} so causal work
is balanced.  Both parities run the SAME program with unified per-key-block
suffix widths (max over parities).

On-chip layout is fully transposed ("T" = [feature, seq]); all attention
operands are f16 (full-rate PE for any tile size, 2x/4x DVE modes), PSUM
accumulation f32.  K/V flow in 512-column chunks overlapped with input DMA.
"""

import numpy as np

B, S, D, H, DK, DV = 4, 2048, 512, 8, 64, 64
NB = S // 128  # 16 key/query blocks per batch
QB = 8  # query blocks per core
SQ = QB * 128  # 1024 query rows per core
N_CORES = 8

# per-parity query block sets (pairs {i, 15-i} -> equal causal work 68)
BLOCKS = {
    0: [0, 2, 4, 6, 9, 11, 13, 15],
    1: [1, 3, 5, 7, 8, 10, 12, 14],
}
# unified suffix width (in 128-blocks) for key-block j = max over parities of
# count of local query blocks with global index >= j
WIDTHS = [
    max(sum(1 for g in BLOCKS[p] if g >= j) for p in (0, 1)) for j in range(NB)
]

# PV accumulator regions: (col_lo, col_hi, psum_tile_idx, tile_col_off, last_j)
PO_R = [(0, 512, 0, 0, 7), (512, 896, 1, 0, 13), (896, 1024, 1, 384, 15)]
# output chunks per region (global 128-col chunk indices)
R_CHUNKS = [(0, 1, 2, 3), (4, 5, 6), (7,)]

# cst (f16) column layout: unduplicated 4-chunk Wq / Wk / Wv, then tri
C_WQ = 0
C_WK = 256
C_WV = 512
C_TRI = 768         # causal tri plane tri[ks, qs] = (qs >= ks)
C_NCOL = 896

N_WARM = 5  # PE p-state warm-up matmuls (512 cols each)


def _build(reps=1, dbg=False):
    import concourse.mybir as mybir
    import concourse.tile as tile
    from concourse import bacc

    F32 = mybir.dt.float32
    F16 = mybir.dt.float16

    nc = bacc.Bacc("TRN2", target_bir_lowering=False, debug=False, num_devices=N_CORES)
    d_qT = nc.dram_tensor("qT", [D, SQ], F16, kind="ExternalInput").ap()
    d_kT = nc.dram_tensor("kT", [D, S], F16, kind="ExternalInput").ap()
    d_cst = nc.dram_tensor("cst", [128, C_NCOL], F16, kind="ExternalInput").ap()
    d_wo = nc.dram_tensor("wo", [DV, D], F16, kind="ExternalInput").ap()
    d_msk = nc.dram_tensor("msk", [128, 2 * NB + 1], F32, kind="ExternalInput").ap()
    d_out = nc.dram_tensor("out", [SQ, D], F16, kind="ExternalOutput").ap()

    for _ in range(reps):
        _emit_body(nc, tile, mybir, d_qT, d_kT, d_cst, d_wo, d_msk, d_out)
    nc.compile()
    return nc


def _emit_body(nc, tile, mybir, d_qT, d_kT, d_cst, d_wo, d_msk, d_out):
    F32 = mybir.dt.float32
    F16 = mybir.dt.float16
    AF = mybir.ActivationFunctionType
    ALU = mybir.AluOpType

    with (
        tile.TileContext(nc) as tc,
        nc.allow_low_precision(reason="f16 attention kernel"),
    ):
        with (
            tc.tile_pool(name="const", bufs=1) as cpool,
            tc.tile_pool(name="acts", bufs=1) as apool,
            tc.tile_pool(name="work", bufs=3) as wpool,
            tc.tile_pool(name="outb", bufs=8) as opool,
            tc.tile_pool(name="psmisc", bufs=2, space="PSUM") as psm,
            tc.tile_pool(name="pscore", bufs=2, space="PSUM") as psc,
            tc.tile_pool(name="pacc", bufs=1, space="PSUM") as pacc,
        ):
            # ---- PE warm-up starts immediately (DVE memset -> zero matmuls)
            # so the modeled p-state ramp completes during the input DMAs ----
            warm = cpool.tile([128, 512], F16)
            nc.vector.memset(warm[:], 0.0)
            wps = psm.tile([128, 512], F32, tag="pp", name="wps")
            for _ in range(N_WARM):
                nc.tensor.matmul(
                    wps[:], warm[:, 0:128], warm[:],
                    start=True, stop=True, skip_group_check=True,
                )

            # ---- input DMAs (order = transfer order on the shared DMA
            # rsrc): Wq first (tiny), then q in 256-col chunks so the first
            # qproj starts early, then the other weights and the k chunks ----
            cst_t = cpool.tile([128, C_NCOL], F16)
            nc.sync.dma_start(cst_t[:, 0:256], d_cst[:, 0:256])
            qT_act = apool.tile([128, 4, SQ], F16)
            qT_r = d_qT.rearrange("(c p) s -> p c s", p=128)
            kT_r = d_kT.rearrange("(c p) s -> p c s", p=128)
            nc.sync.dma_start(qT_act[:, :, 0:512], qT_r[:, :, 0:512])
            nc.sync.dma_start(qT_act[:, :, 512:1024], qT_r[:, :, 512:1024])
            msk_t = cpool.tile([128, 2 * NB + 1], F32)
            nc.sync.dma_start(msk_t[:], d_msk[:])
            nc.sync.dma_start(cst_t[:, 256:C_NCOL], d_cst[:, 256:C_NCOL])
            kT_act = apool.tile([128, 4, S], F16)
            nc.sync.dma_start(kT_act[:, :, 0:512], kT_r[:, :, 0:512])
            wo_t = cpool.tile([DV, D], F16)
            nc.sync.dma_start(wo_t[:], d_wo[:])
            for n in range(1, 4):
                sl = slice(512 * n, 512 * (n + 1))
                nc.sync.dma_start(kT_act[:, :, sl], kT_r[:, :, sl])

            id1 = msk_t[0:1, 2 * NB : 2 * NB + 1]
            # per-j mask planes {tri | ones | zeros} synthesized as
            # tri * b_j + a_j with per-core selector scalars; runs on idle
            # DVE time during the input-DMA window.
            tri_t = cst_t[:, C_TRI : C_TRI + 128]
            bm16 = apool.tile([128, NB, 128], F16)
            for j in range(NB):
                nc.vector.tensor_scalar(
                    bm16[:, j, :], tri_t[:],
                    msk_t[:, j : j + 1], msk_t[:, NB + j : NB + j + 1],
                    ALU.mult, ALU.add,
                )

            # ---- persistent SBUF tensors ----
            qT_proj = apool.tile([DK, SQ], F16)
            kT_proj = apool.tile([DK, S], F16)
            v1 = apool.tile([128, NB, DV + 1], F16)
            nc.vector.memset(v1[:, :, DV : DV + 1], 1.0)
            oT = apool.tile([DV, SQ], F16)
            srow = apool.tile([1, SQ], F32)
            recipT = apool.tile([128, QB], F32)
            po = [
                pacc.tile([DV + 1, 512], F32, name="po0"),
                pacc.tile([DV + 1, 512], F32, name="po12"),
            ]

            def qproj(n):
                sl = slice(512 * n, 512 * (n + 1))
                ps = psm.tile([128, 512], F32, tag="pp", name="psq")
                for c in range(4):
                    w = cst_t[:, C_WQ + 64 * c : C_WQ + 64 * c + 64]
                    nc.tensor.matmul(
                        ps[0:DK, :], w, qT_act[:, c, sl],
                        start=(c == 0), stop=(c == 3),
                    )
                nc.scalar.copy(qT_proj[:, sl], ps[0:DK, :])

            def kvproj(n):
                sl = slice(512 * n, 512 * (n + 1))
                ps = psm.tile([128, 512], F32, tag="pp", name="psk")
                for c in range(4):
                    w = cst_t[:, C_WK + 64 * c : C_WK + 64 * c + 64]
                    nc.tensor.matmul(
                        ps[0:DK, :], w, kT_act[:, c, sl],
                        start=(c == 0), stop=(c == 3),
                    )
                nc.vector.tensor_copy(kT_proj[:, sl], ps[0:DK, :])
                # v computed directly row-major: out partitions = tokens, so
                # no PE transpose / vT staging is needed.  4 token-blocks per
                # chunk; groups must stay contiguous (start=True marks the
                # tile's whole psum bank pending-zero).
                ps = psm.tile([128, 512], F32, tag="pp", name="psv")
                for jb in range(4):
                    tok = slice(512 * n + 128 * jb, 512 * n + 128 * jb + 128)
                    for c in range(4):
                        w = cst_t[:, C_WV + 64 * c : C_WV + 64 * c + 64]
                        nc.tensor.matmul(
                            ps[:, 64 * jb : 64 * jb + 64],
                            kT_act[:, c, tok], w[:],
                            start=(c == 0), stop=(c == 3),
                            skip_group_check=True,
                        )
                nc.vector.tensor_copy(v1[:, 4 * n : 4 * n + 4, 0:DV], ps[:, 0:256])

            exps = {}

            def scores(*js):
                # one psc tile + one exp activation shared by the (narrow)
                # key-blocks js; per-j mask multiplies on each first block
                ps_s = psc.tile([128, 1024], F32, tag="pss")
                expT = wpool.tile([128, 1024], F16, tag="expT", bufs=6)
                off = 0
                offs = []
                for j in js:
                    wblk = WIDTHS[j]
                    c0 = 128 * (QB - wblk)
                    cols = 128 * wblk
                    done = 0
                    while done < cols:
                        # matmul writes must not cross a psum bank boundary
                        nsz = min(512 - (off + done) % 512, cols - done)
                        nc.tensor.matmul(
                            ps_s[:, off + done : off + done + nsz],
                            kT_proj[:, 128 * j : 128 * (j + 1)],
                            qT_proj[:, c0 + done : c0 + done + nsz],
                            start=True, stop=True,
                        )
                        done += nsz
                    offs.append((j, off, cols))
                    off += cols
                nc.scalar.activation(
                    expT[:, 0:off], ps_s[:, 0:off], AF.Exp, bias=0.0, scale=0.125
                )
                for j, o, cols in offs:
                    nc.vector.tensor_mul(
                        expT[:, o : o + 128], expT[:, o : o + 128], bm16[:, j, :]
                    )
                    exps[j] = expT[:, o : o + cols]

            def emit_pv(j, masked):
                # masked=False: bulk columns (depend on exp only);
                # masked=True: the first 128-col block (awaits the mask mul).
                # start=True marks the tile's WHOLE 2KB psum bank pending-
                # zero, so only the first write of each bank carries it; the
                # bank-sharing region 2 and the masked block are zero-filled
                # by the pending state instead.
                wblk = WIDTHS[j]
                c0 = 128 * (QB - wblk)
                for rlo, rhi, ti, toff, lastj in PO_R:
                    lo = max(c0, rlo)
                    if lo >= rhi:
                        continue
                    if lo == c0:
                        a, b = (c0, c0 + 128) if masked else (c0 + 128, rhi)
                    else:
                        if masked:
                            continue
                        a, b = lo, rhi
                    if a >= b:
                        continue
                    nc.tensor.matmul(
                        po[ti][:, toff + a - rlo : toff + b - rlo],
                        v1[:, j, :],
                        exps[j][:, a - c0 : b - c0],
                        start=(j == 0 and toff == 0 and not masked),
                        stop=(j == lastj),
                        skip_group_check=True,
                    )

            def pv(j):
                emit_pv(j, False)
                emit_pv(j, True)

            def ep_copy(q):
                rlo, rhi, ti, toff, _ = PO_R[q]
                w = rhi - rlo
                nc.scalar.copy(oT[:, rlo:rhi], po[ti][0:DV, toff : toff + w])
                nc.vector.tensor_copy(
                    srow[:, rlo:rhi], po[ti][DV : DV + 1, toff : toff + w]
                )

            def ep_den(q):
                chunks = R_CHUNKS[q]
                pd = psm.tile([128, 512], F32, tag="pp", name="pd")
                for ci, g in enumerate(chunks):
                    nc.tensor.transpose(
                        pd[:, ci : ci + 1],
                        srow[0:1, 128 * g : 128 * (g + 1)],
                        id1,
                    )
                g0 = chunks[0]
                nc.vector.reciprocal(
                    recipT[:, g0 : g0 + len(chunks)], pd[:, 0 : len(chunks)]
                )

            def ep_out(q, i):
                g = R_CHUNKS[q][i]
                pf = psm.tile([128, 512], F32, tag="pp", name="pf")
                nc.tensor.matmul(
                    pf[:], oT[:, 128 * g : 128 * (g + 1)], wo_t[:],
                    start=True, stop=True,
                )
                osb = opool.tile([128, D], F16, tag="osb")
                if g % 2 == 0:
                    nc.vector.tensor_scalar_mul(osb[:], pf[:], recipT[:, g : g + 1])
                else:
                    nc.scalar.activation(
                        osb[:], pf[:], mybir.ActivationFunctionType.Copy,
                        bias=0.0, scale=recipT[:, g : g + 1],
                    )
                eng = nc.sync if g % 2 == 0 else nc.scalar
                eng.dma_start(d_out[128 * g : 128 * (g + 1), :], osb[:])

            # ---- schedule ----
            import os as _os
            _sched = _os.environ.get("K_SCHED", "S1")
            qproj(0)
            qproj(1)
            kvproj(0)
            kvproj(1)
            for j in range(8):
                if j == 4:
                    kvproj(2)
                if j == 7 and _sched == "S3":
                    kvproj(3)
                scores(j)
                if j >= 1:
                    pv(j - 1)
            scores(8)
            scores(9)
            pv(7)
            ep_copy(0)
            if _sched != "S3":
                kvproj(3)
            scores(10)
            scores(11)
            pv(8)
            pv(9)
            ep_den(0)
            ep_out(0, 0)
            scores(12)
            scores(13)
            pv(10)
            pv(11)
            ep_out(0, 1)
            ep_out(0, 2)
            scores(14, 15)
            pv(12)
            pv(13)
            ep_out(0, 3)
            ep_copy(1)
            ep_den(1)
            ep_out(1, 0)
            pv(14)
            ep_out(1, 1)
            pv(15)
            ep_copy(2)
            ep_den(2)
            ep_out(2, 0)
            ep_out(1, 2)


_NC_CACHE = None


def _get_nc():
    global _NC_CACHE
    if _NC_CACHE is None:
        _NC_CACHE = _build()
    return _NC_CACHE


def make_in_maps(query, key, Wq, Wk, Wv, Wo):
    query = np.asarray(query, dtype=np.float32)
    key = np.asarray(key, dtype=np.float32)
    Wq = np.asarray(Wq, dtype=np.float32)
    Wk = np.asarray(Wk, dtype=np.float32)
    Wv = np.asarray(Wv, dtype=np.float32)
    Wo = np.asarray(Wo, dtype=np.float32)

    wo_r = Wo.reshape(H, DV, D).sum(axis=0).astype(np.float16)  # [DV, D]
    cst = np.zeros((128, C_NCOL), np.float16)
    for w, base in ((Wq, C_WQ), (Wk, C_WK), (Wv, C_WV)):
        cst[:, base : base + 256] = (
            w.astype(np.float16).reshape(4, 128, 64).transpose(1, 0, 2).reshape(128, 256)
        )
    cst[:, C_TRI : C_TRI + 128] = np.triu(np.ones((128, 128), np.float16))

    in_maps = []
    for c in range(N_CORES):
        b, p = divmod(c, 2)
        blocks = BLOCKS[p]
        rows = np.concatenate(
            [np.arange(128 * g, 128 * (g + 1)) for g in blocks]
        )
        qT = np.ascontiguousarray(query[b][rows].T).astype(np.float16)
        kT = np.ascontiguousarray(key[b].T).astype(np.float16)
        msk = np.zeros((128, 2 * NB + 1), np.float32)
        for j in range(NB):
            g = blocks[QB - WIDTHS[j]]
            if g == j:
                msk[:, j] = 1.0  # b: tri plane
            elif g > j:
                msk[:, NB + j] = 1.0  # a: all-ones plane
            # g < j: both zero -> all-zeros plane
        msk[:, 2 * NB] = 1.0  # 1x1 identity for the denom row transposes
        in_maps.append({"qT": qT, "kT": kT, "cst": cst, "wo": wo_r, "msk": msk})
    return in_maps


def gather_output(results, bias_term):
    """results: list of per-core {'out': [SQ, D]}; adds host-folded bias."""
    out = np.empty((B, S, D), np.float32)
    for c in range(N_CORES):
        b, p = divmod(c, 2)
        blocks = BLOCKS[p]
        co = np.asarray(results[c]["out"], dtype=np.float32)
        for bp, g in enumerate(blocks):
            out[b, 128 * g : 128 * (g + 1), :] = co[128 * bp : 128 * (bp + 1), :]
    out += bias_term
    return out


def kernel(query, key, value, Wq, bq, Wk, bk, Wv, bv, Wo, bo):
    from concourse import bass_utils

    nc = _get_nc()
    in_maps = make_in_maps(query, key, Wq, Wk, Wv, Wo)
    res = bass_utils.run_bass_kernel_spmd(
        nc, in_maps, core_ids=list(range(N_CORES))
    )
    Wo = np.asarray(Wo, dtype=np.float32)
    wo_r = Wo.reshape(H, DV, D).sum(axis=0)
    # bq/bk only shift scores per query row (softmax-invariant); bv adds
    # bv @ Wo_r to every output row since attention rows sum to 1.
    bias_term = np.asarray(bv, np.float32) @ wo_r + np.asarray(bo, np.float32)
    return gather_output(res.results, bias_term.astype(np.float32))
